# revision 20
# baseline (speedup 1.0000x reference)
"""Trainium2 Bass kernel for the Gumbel-softmax tokenizer.

Sharding: data-parallel over B=8 event batches, one batch per NeuronCore.
Per core (M=8192 points):
  - per-point MLP 32->256->512->768 in transposed orientation (features on
    partitions, points on free dim), fp32 matmuls
  - selection scores via folded sel_w@W4^T against h3 (per-slot constant
    sel_w@b4 dropped: it cannot change a per-row argmax)
  - Gumbel noise is a data-independent constant (jax key 42), precomputed
    on host; greedy no-replacement argmax scan realized as Max8 top-8
    candidates per slot + a Jacobi fixpoint repair (collisions are ~1/batch)
  - h3 stored to DRAM in natural orientation (PE transposes); KNN-16 via
    Max8/MatchReplace on -d2/2 rows, neighbor rows gathered with indirect
    DMA, W4 applied post-gather, max-pool, 2-layer MLP, add
  - tokens sorted by time with a rank-onehot permutation matmul
"""
import functools
import numpy as np

B = 8
M = 8192
K = 128
KN = 16
CH = 512          # point chunk for the MLP phase
NCH = M // CH     # 16
D1, D2, D3 = 256, 512, 768
F32 = None        # set in _build (mybir.dt.float32)

R_JACOBI = 8


@functools.cache
def _gumbel_np():
    import jax
    import jax.numpy as jnp
    with jax.default_device(jax.devices("cpu")[0]):
        u = jax.random.uniform(jax.random.key(42), (B, K, M),
                               dtype=jnp.float32, minval=1e-20, maxval=1.0)
        g = -jnp.log(-jnp.log(u))
        return np.asarray(jax.device_get(g)).astype(np.float32)


def _ktile_wide(w: np.ndarray) -> np.ndarray:
    """[kt*128, n] -> [128, kt*n] with block kk = w[kk*128:(kk+1)*128, :]."""
    kin, n = w.shape
    kt = kin // 128
    return np.ascontiguousarray(
        w.reshape(kt, 128, n).transpose(1, 0, 2).reshape(128, kt * n)
    ).astype(np.float32)


@functools.cache
def _build():
    import concourse.bacc as bacc
    import concourse.bass as bass
    import concourse.mybir as mybir
    import concourse.tile as tile
    from concourse.alu_op_type import AluOpType

    f32 = mybir.dt.float32
    u32 = mybir.dt.uint32
    AF = mybir.ActivationFunctionType

    nc = bacc.Bacc("TRN2", target_bir_lowering=False, debug=False,
                   num_devices=B)

    # ---- DRAM I/O ----
    # featsT packed: quarter g of the point axis lives on partitions 32g:32g+32
    featsT_d = nc.dram_tensor("featsT", [128, M // 4], f32, kind="ExternalInput").ap()
    # coordsT4 packed: quarter g on partitions 32g:32g+4 (rest zero-padded)
    coordsT4_d = nc.dram_tensor("coordsT4", [128, M // 4], f32, kind="ExternalInput").ap()
    ptsc_d = nc.dram_tensor("ptsc", [M, 4], f32, kind="ExternalInput").ap()
    gumb_d = nc.dram_tensor("gumb", [K, M], f32, kind="ExternalInput").ap()
    w1_d = nc.dram_tensor("w1", [128, D1], f32, kind="ExternalInput").ap()  # W1 tiled 4x
    w2_d = nc.dram_tensor("w2", [128, 2 * D2], f32, kind="ExternalInput").ap()
    w3_d = nc.dram_tensor("w3", [128, 4 * D3], f32, kind="ExternalInput").ap()
    w4_d = nc.dram_tensor("w4", [128, 6 * D3], f32, kind="ExternalInput").ap()
    selw_d = nc.dram_tensor("selw", [128, 6 * K], f32, kind="ExternalInput").ap()
    nw1_d = nc.dram_tensor("nw1", [128, 6 * D3], f32, kind="ExternalInput").ap()
    nw2_d = nc.dram_tensor("nw2", [128, 6 * D3], f32, kind="ExternalInput").ap()
    b1_d = nc.dram_tensor("b1c", [128, 2], f32, kind="ExternalInput").ap()
    b2_d = nc.dram_tensor("b2c", [128, 4], f32, kind="ExternalInput").ap()
    b3_d = nc.dram_tensor("b3c", [128, 6], f32, kind="ExternalInput").ap()
    b4r_d = nc.dram_tensor("b4row", [1, D3], f32, kind="ExternalInput").ap()
    nb1r_d = nc.dram_tensor("nb1row", [1, D3], f32, kind="ExternalInput").ap()
    nb2r_d = nc.dram_tensor("nb2row", [1, D3], f32, kind="ExternalInput").ap()
    ident_d = nc.dram_tensor("ident", [128, 128], f32, kind="ExternalInput").ap()
    iota8_d = nc.dram_tensor("iota8t", [128, 8], f32, kind="ExternalInput").ap()
    iota128_d = nc.dram_tensor("iota128t", [128, 128], f32, kind="ExternalInput").ap()
    lt128_d = nc.dram_tensor("lt128", [128, 128], f32, kind="ExternalInput").ap()

    otok_d = nc.dram_tensor("otok", [K, D3], f32, kind="ExternalOutput").ap()
    ocent_d = nc.dram_tensor("ocent", [K, 4], f32, kind="ExternalOutput").ap()
    h3nat_d = nc.dram_tensor("h3nat", [M, D3], f32).ap()   # internal scratch

    with tile.TileContext(nc) as tc:
        with tc.tile_pool(name="cst", bufs=1) as cst, \
             tc.tile_pool(name="big", bufs=1) as big, \
             tc.tile_pool(name="wrk", bufs=2) as wrk, \
             tc.tile_pool(name="h3p", bufs=1) as h3p, \
             tc.tile_pool(name="wr1", bufs=1) as wr1, \
             tc.tile_pool(name="h3n", bufs=2) as h3n, \
             tc.tile_pool(name="sml", bufs=1) as sml, \
             tc.tile_pool(name="psA", bufs=3, space="PSUM") as psA, \
             tc.tile_pool(name="psS", bufs=2, space="PSUM") as psS, \
             tc.tile_pool(name="psT", bufs=2, space="PSUM") as psT:

            def load(pool, dram, shape, tag):
                t = pool.tile(shape, f32, tag=tag)
                nc.sync.dma_start(t[:], dram[:])
                return t

            featsT = load(cst, featsT_d, [128, M // 4], "featsT")
            coordsT4 = load(cst, coordsT4_d, [128, M // 4], "coordsT4")
            w1 = load(cst, w1_d, [128, D1], "w1")
            w2 = load(cst, w2_d, [128, 2 * D2], "w2")
            w3 = load(cst, w3_d, [128, 4 * D3], "w3")
            w4 = load(cst, w4_d, [128, 6 * D3], "w4")
            selw = load(cst, selw_d, [128, 6 * K], "selw")
            nw1 = load(cst, nw1_d, [128, 6 * D3], "nw1")
            nw2 = load(cst, nw2_d, [128, 6 * D3], "nw2")
            b1c = load(cst, b1_d, [128, 2], "b1c")
            b2c = load(cst, b2_d, [128, 4], "b2c")
            b3c = load(cst, b3_d, [128, 6], "b3c")
            b4row = load(cst, b4r_d, [1, D3], "b4row")
            nb1row = load(cst, nb1r_d, [1, D3], "nb1row")
            nb2row = load(cst, nb2r_d, [1, D3], "nb2row")
            ident = load(cst, ident_d, [128, 128], "ident")
            iota8t = load(cst, iota8_d, [128, 8], "iota8t")
            iota128t = load(cst, iota128_d, [128, 128], "iota128t")
            lt128 = load(cst, lt128_d, [128, 128], "lt128")
            ones1 = cst.tile([1, 128], f32, tag="ones1")
            nc.vector.memset(ones1[:], 1.0)

            # A starts as the gumbel noise; scores are accumulated into it
            A = big.tile([K, M], f32, tag="bigmat")
            nc.sync.dma_start(A[:], gumb_d[:])

            # ---------------- phase 1: MLP + scores + h3 natural ----------
            for n in range(NCH):
                fs = slice(n * CH, (n + 1) * CH)
                h1c = wrk.tile([128, 2 * CH], f32, tag="h1c")
                fg = n // 4                       # packed quarter of featsT
                fof = (n % 4) * CH                # column offset inside quarter
                for fo in range(2):
                    ps = psA.tile([128, CH], f32, tag="psA")
                    nc.tensor.matmul(ps[:],
                                     w1[32 * fg:32 * (fg + 1), fo * 128:(fo + 1) * 128],
                                     featsT[32 * fg:32 * (fg + 1), fof:fof + CH],
                                     start=True, stop=True,
                                     tile_position=(32 * fg, 0))
                    nc.scalar.activation(h1c[:, fo * CH:(fo + 1) * CH], ps[:],
                                         AF.Relu, bias=b1c[:, fo:fo + 1])
                h2c = wr1.tile([128, 4 * CH], f32, tag="h2c")
                for fo in range(4):
                    ps = psA.tile([128, CH], f32, tag="psA")
                    for kk in range(2):
                        nc.tensor.matmul(
                            ps[:],
                            w2[:, kk * D2 + fo * 128: kk * D2 + (fo + 1) * 128],
                            h1c[:, kk * CH:(kk + 1) * CH],
                            start=(kk == 0), stop=(kk == 1))
                    nc.scalar.activation(h2c[:, fo * CH:(fo + 1) * CH], ps[:],
                                         AF.Relu, bias=b2c[:, fo:fo + 1])
                h3c = h3p.tile([128, 6 * CH], f32, tag="h3c")
                for fo in range(6):
                    ps = psA.tile([128, CH], f32, tag="psA")
                    for kk in range(4):
                        nc.tensor.matmul(
                            ps[:],
                            w3[:, kk * D3 + fo * 128: kk * D3 + (fo + 1) * 128],
                            h2c[:, kk * CH:(kk + 1) * CH],
                            start=(kk == 0), stop=(kk == 3))
                    nc.scalar.activation(h3c[:, fo * CH:(fo + 1) * CH], ps[:],
                                         AF.Relu, bias=b3c[:, fo:fo + 1])
                # scores for this chunk: A[:, fs] += selw_fold^T @ h3
                ps_sc = psS.tile([K, CH], f32, tag="psS")
                for kk in range(6):
                    nc.tensor.matmul(ps_sc[:], selw[:, kk * K:(kk + 1) * K],
                                     h3c[:, kk * CH:(kk + 1) * CH],
                                     start=(kk == 0), stop=(kk == 5))
                nc.vector.scalar_tensor_tensor(
                    out=A[:, fs], in0=ps_sc[:], scalar=1.0, in1=A[:, fs],
                    op0=AluOpType.mult, op1=AluOpType.add)
                # h3 natural chunks -> DRAM
                for j in range(CH // 128):
                    hn = h3n.tile([128, D3], f32, tag="h3natc")
                    for fo in range(6):
                        pt = psT.tile([128, 128], f32, tag="psT")
                        nc.tensor.transpose(
                            pt[:], h3c[:, fo * CH + j * 128: fo * CH + (j + 1) * 128],
                            ident[:])
                        eng = nc.scalar if (fo % 2 == 0) else nc.vector
                        if fo % 2 == 0:
                            nc.scalar.activation(
                                hn[:, fo * 128:(fo + 1) * 128], pt[:], AF.Copy)
                        else:
                            nc.vector.tensor_copy(
                                hn[:, fo * 128:(fo + 1) * 128], pt[:])
                    nc.sync.dma_start(
                        h3nat_d[n * CH + j * 128: n * CH + (j + 1) * 128, :],
                        hn[:])

            # ---------------- phase 2: greedy scan via top-8 + Jacobi -----
            cand_v = sml.tile([K, 8], f32, tag="cand_v")
            cand_u = sml.tile([K, 8], u32, tag="cand_u")
            cand_f = sml.tile([K, 8], f32, tag="cand_f")
            nc.vector.max(cand_v[:], A[:])
            nc.vector.max_index(cand_u[:], cand_v[:], A[:])
            nc.vector.tensor_copy(cand_f[:], cand_u[:])
            picks_f = sml.tile([K, 1], f32, tag="picks_f")
            nc.vector.tensor_copy(picks_f[:], cand_f[:, 0:1])
            eqbuf = sml.tile([K, 8 * 128], f32, tag="eqbuf")
            badc = sml.tile([K, 8], f32, tag="badc")
            t8 = sml.tile([K, 8], f32, tag="t8")
            ptr = sml.tile([K, 1], f32, tag="ptr")
            oh8 = sml.tile([K, 8], f32, tag="oh8")
            scr8 = sml.tile([K, 8], f32, tag="scr8")
            prow = sml.tile([1, 128], f32, tag="prow")
            for _ in range(R_JACOBI):
                pt = psT.tile([128, 128], f32, tag="psT")
                nc.tensor.transpose(pt[0:1, :], picks_f[:], ident[:])
                nc.scalar.activation(prow[:], pt[0:1, :], AF.Copy)
                pb = psA.tile([128, CH], f32, tag="psA")
                nc.tensor.matmul(pb[:, 0:128], ones1[:], prow[:],
                                 start=True, stop=True)
                nc.vector.tensor_tensor(
                    out=eqbuf[:].rearrange("p (c j) -> p c j", c=8),
                    in0=cand_f[:].unsqueeze(2).to_broadcast([K, 8, 128]),
                    in1=pb[:, 0:128].unsqueeze(1).to_broadcast([K, 8, 128]),
                    op=AluOpType.is_equal)
                nc.vector.tensor_tensor(
                    out=eqbuf[:].rearrange("p (c j) -> p c j", c=8),
                    in0=eqbuf[:].rearrange("p (c j) -> p c j", c=8),
                    in1=lt128[:].unsqueeze(1).to_broadcast([K, 8, 128]),
                    op=AluOpType.mult)
                nc.vector.tensor_reduce(
                    out=badc[:], in_=eqbuf[:].rearrange("p (c j) -> p c j", c=8),
                    axis=mybir.AxisListType.X, op=AluOpType.add)
                nc.vector.scalar_tensor_tensor(
                    out=t8[:], in0=badc[:], scalar=1e9, in1=iota8t[:],
                    op0=AluOpType.mult, op1=AluOpType.add)
                nc.vector.tensor_reduce(out=ptr[:], in_=t8[:],
                                        axis=mybir.AxisListType.X,
                                        op=AluOpType.min)
                nc.vector.tensor_scalar(
                    out=oh8[:], in0=iota8t[:], scalar1=ptr[:, 0:1], scalar2=None,
                    op0=AluOpType.is_equal)
                nc.vector.tensor_tensor(out=scr8[:], in0=oh8[:],
                                        in1=cand_f[:], op=AluOpType.mult)
                nc.vector.tensor_reduce(out=picks_f[:], in_=scr8[:],
                                        axis=mybir.AxisListType.X,
                                        op=AluOpType.add)
            picks_u = sml.tile([K, 1], u32, tag="picks_u")
            nc.vector.tensor_copy(picks_u[:], picks_f[:])

            # ---------------- phase 3: gather selected, KNN ---------------
            h3sel = sml.tile([K, D3], f32, tag="h3sel")
            nc.gpsimd.indirect_dma_start(
                out=h3sel[:], out_offset=None, in_=h3nat_d[:],
                in_offset=bass.IndirectOffsetOnAxis(ap=picks_u[:, 0:1], axis=0))
            csel = sml.tile([K, 4], f32, tag="csel")
            nc.gpsimd.indirect_dma_start(
                out=csel[:], out_offset=None, in_=ptsc_d[:],
                in_offset=bass.IndirectOffsetOnAxis(ap=picks_u[:, 0:1], axis=0))
            ct = psT.tile([128, 128], f32, tag="psT")
            nc.tensor.transpose(ct[0:4, :], csel[:], ident[:])
            # lhsT for d2: rows 32g:32g+3 = [x;y;z]; every other row 1.0 —
            # the matching rhs (coordsT4) rows are -|p|^2/2 at 32g+3 and
            # zero-padding elsewhere, so the ones are harmless.
            lhsT4 = sml.tile([128, 128], f32, tag="lhsT4")
            trow = sml.tile([1, 128], f32, tag="trow")
            nc.vector.memset(lhsT4[:], 1.0)
            for g in range(4):
                nc.vector.tensor_copy(lhsT4[32 * g:32 * g + 3, :], ct[0:3, :])
            ct2 = psT.tile([128, 128], f32, tag="psT")
            nc.tensor.transpose(ct2[0:1, :], csel[:, 3:4], ident[:])
            nc.vector.tensor_copy(trow[:], ct2[0:1, :])

            Dm = big.tile([K, M], f32, tag="bigmat")
            for n in range(NCH):
                fs = slice(n * CH, (n + 1) * CH)
                g = n // 4
                fof = (n % 4) * CH
                ps = psA.tile([128, CH], f32, tag="psA")
                nc.tensor.matmul(ps[:],
                                 lhsT4[32 * g:32 * (g + 1), :],
                                 coordsT4[32 * g:32 * (g + 1), fof:fof + CH],
                                 start=True, stop=True,
                                 tile_position=(32 * g, 0))
                if n % 2 == 0:
                    nc.scalar.activation(Dm[:, fs], ps[:], AF.Copy)
                else:
                    nc.vector.tensor_copy(Dm[:, fs], ps[:])
            knn_u = sml.tile([K, KN], u32, tag="knn_u")
            r1v = sml.tile([K, 8], f32, tag="r1v")
            r2v = sml.tile([K, 8], f32, tag="r2v")
            nc.vector.max(r1v[:], Dm[:])
            nc.vector.max_index(knn_u[:, 0:8], r1v[:], Dm[:])
            nc.vector.match_replace(Dm[:], r1v[:], Dm[:], -3e38)
            nc.vector.max(r2v[:], Dm[:])
            nc.vector.max_index(knn_u[:, 8:16], r2v[:], Dm[:])

            # ---------------- phase 4: W4 on gathered rows, pool, MLP -----
            def w4_apply(src_T, ps_a, ps_b, bias_row):
                """psum[K, D3] (split 512+256) = src_T^T @ W4-like + bias."""
                for kk in range(6):
                    nc.tensor.matmul(
                        ps_a[:], src_T[:, kk * 128:(kk + 1) * 128],
                        w4[:, kk * D3: kk * D3 + 512],
                        start=(kk == 0), stop=False)
                    nc.tensor.matmul(
                        ps_b[:, 0:256], src_T[:, kk * 128:(kk + 1) * 128],
                        w4[:, kk * D3 + 512: (kk + 1) * D3],
                        start=(kk == 0), stop=False)
                nc.tensor.matmul(ps_a[:], ones1[:], bias_row[0:1, 0:512],
                                 start=False, stop=True)
                nc.tensor.matmul(ps_b[:, 0:256], ones1[:], bias_row[0:1, 512:D3],
                                 start=False, stop=True)

            def transpose6(src, tag):
                dst = wr1.tile([128, 6 * 128], f32, tag=tag)
                for fo in range(6):
                    pt = psT.tile([128, 128], f32, tag="psT")
                    nc.tensor.transpose(
                        pt[:], src[:, fo * 128:(fo + 1) * 128], ident[:])
                    if fo % 2 == 0:
                        nc.scalar.activation(
                            dst[:, fo * 128:(fo + 1) * 128], pt[:], AF.Copy)
                    else:
                        nc.vector.tensor_copy(
                            dst[:, fo * 128:(fo + 1) * 128], pt[:])
                return dst

            pooled = sml.tile([K, D3], f32, tag="pooled")
            for k in range(KN):
                gk = wrk.tile([K, D3], f32, tag="gk")
                nc.gpsimd.indirect_dma_start(
                    out=gk[:], out_offset=None, in_=h3nat_d[:],
                    in_offset=bass.IndirectOffsetOnAxis(
                        ap=knn_u[:, k:k + 1], axis=0))
                gkT = transpose6(gk, "gkT")
                ps_a = psA.tile([128, CH], f32, tag="psA")
                ps_b = psA.tile([128, CH], f32, tag="psA")
                w4_apply(gkT, ps_a, ps_b, b4row)
                if k == 0:
                    nc.scalar.activation(pooled[:, 0:512], ps_a[:], AF.Copy)
                    nc.scalar.activation(pooled[:, 512:D3], ps_b[:, 0:256], AF.Copy)
                else:
                    nc.vector.tensor_max(pooled[:, 0:512], pooled[:, 0:512], ps_a[:])
                    nc.vector.tensor_max(pooled[:, 512:D3], pooled[:, 512:D3],
                                         ps_b[:, 0:256])

            h3selT = transpose6(h3sel, "gkT")
            ps_a = psA.tile([128, CH], f32, tag="psA")
            ps_b = psA.tile([128, CH], f32, tag="psA")
            w4_apply(h3selT, ps_a, ps_b, b4row)
            h4sel = sml.tile([K, D3], f32, tag="h4sel")
            nc.scalar.activation(h4sel[:, 0:512], ps_a[:], AF.Copy)
            nc.scalar.activation(h4sel[:, 512:D3], ps_b[:, 0:256], AF.Copy)

            def lin768(src_nat, wwide, bias_row, tag, relu):
                srcT = transpose6(src_nat, tag)
                ps_a = psA.tile([128, CH], f32, tag="psA")
                ps_b = psA.tile([128, CH], f32, tag="psA")
                for kk in range(6):
                    nc.tensor.matmul(
                        ps_a[:], srcT[:, kk * 128:(kk + 1) * 128],
                        wwide[:, kk * D3: kk * D3 + 512],
                        start=(kk == 0), stop=False)
                    nc.tensor.matmul(
                        ps_b[:, 0:256], srcT[:, kk * 128:(kk + 1) * 128],
                        wwide[:, kk * D3 + 512: (kk + 1) * D3],
                        start=(kk == 0), stop=False)
                nc.tensor.matmul(ps_a[:], ones1[:], bias_row[0:1, 0:512],
                                 start=False, stop=True)
                nc.tensor.matmul(ps_b[:, 0:256], ones1[:],
                                 bias_row[0:1, 512:D3], start=False, stop=True)
                return ps_a, ps_b

            a1a, a1b = lin768(pooled, nw1, nb1row, "gkT", True)
            agg1 = sml.tile([K, D3], f32, tag="pooled")
            nc.scalar.activation(agg1[:, 0:512], a1a[:], AF.Relu)
            nc.scalar.activation(agg1[:, 512:D3], a1b[:, 0:256], AF.Relu)
            a2a, a2b = lin768(agg1, nw2, nb2row, "gkT", False)
            tokens = sml.tile([K, D3], f32, tag="tokens")
            nc.vector.tensor_add(tokens[:, 0:512], h4sel[:, 0:512], a2a[:])
            nc.vector.tensor_add(tokens[:, 512:D3], h4sel[:, 512:D3],
                                 a2b[:, 0:256])

            # ---------------- phase 5: time sort + outputs ----------------
            tb_ps = psA.tile([128, CH], f32, tag="psA")
            nc.tensor.matmul(tb_ps[:, 0:128], ones1[:], trow[:],
                             start=True, stop=True)
            tb = sml.tile([128, 128], f32, tag="tb")
            nc.scalar.activation(tb[:], tb_ps[:, 0:128], AF.Copy)
            lcmp = sml.tile([128, 128], f32, tag="lcmp")
            ecmp = sml.tile([128, 128], f32, tag="ecmp")
            nc.vector.tensor_scalar(
                out=lcmp[:], in0=tb[:], scalar1=csel[:, 3:4], scalar2=None,
                op0=AluOpType.is_lt)
            nc.vector.tensor_scalar(
                out=ecmp[:], in0=tb[:], scalar1=csel[:, 3:4], scalar2=None,
                op0=AluOpType.is_equal)
            nc.vector.tensor_tensor(out=ecmp[:], in0=ecmp[:], in1=lt128[:],
                                    op=AluOpType.mult)
            rank = sml.tile([K, 1], f32, tag="rank")
            scr128 = sml.tile([128, 128], f32, tag="scr128")
            nc.vector.tensor_tensor(out=scr128[:], in0=lcmp[:], in1=ecmp[:],
                                    op=AluOpType.add)
            nc.vector.tensor_reduce(out=rank[:], in_=scr128[:],
                                    axis=mybir.AxisListType.X,
                                    op=AluOpType.add)
            perm = sml.tile([128, 128], f32, tag="perm")
            nc.vector.tensor_scalar(
                out=perm[:], in0=iota128t[:], scalar1=rank[:, 0:1], scalar2=None,
                op0=AluOpType.is_equal)
            so_a = psA.tile([128, CH], f32, tag="psA")
            so_b = psA.tile([128, CH], f32, tag="psA")
            nc.tensor.matmul(so_a[:], perm[:], tokens[:, 0:512],
                             start=True, stop=True)
            nc.tensor.matmul(so_b[:, 0:256], perm[:], tokens[:, 512:D3],
                             start=True, stop=True)
            otok_sb = sml.tile([K, D3], f32, tag="h4sel")
            nc.scalar.activation(otok_sb[:, 0:512], so_a[:], AF.Copy)
            nc.vector.tensor_copy(otok_sb[:, 512:D3], so_b[:, 0:256])
            nc.sync.dma_start(otok_d[:], otok_sb[:])
            sc_ps = psS.tile([K, CH], f32, tag="psS")
            nc.tensor.matmul(sc_ps[:, 0:4], perm[:], csel[:],
                             start=True, stop=True)
            ocent_sb = sml.tile([K, 4], f32, tag="ocent_sb")
            nc.vector.tensor_copy(ocent_sb[:], sc_ps[:, 0:4])
            nc.sync.dma_start(ocent_d[:], ocent_sb[:])

    nc.compile()
    return nc


def _host_prep(inputs):
    coordinates = np.asarray(inputs["coordinates"], np.float32)
    features = np.asarray(inputs["features"], np.float32)
    W1 = np.asarray(inputs["W1"], np.float32)
    W2 = np.asarray(inputs["W2"], np.float32)
    W3 = np.asarray(inputs["W3"], np.float32)
    W4 = np.asarray(inputs["W4"], np.float32)
    sel_w = np.asarray(inputs["sel_w"], np.float32)
    nW1 = np.asarray(inputs["nW1"], np.float32)
    nW2 = np.asarray(inputs["nW2"], np.float32)
    b1 = np.asarray(inputs["b1"], np.float32)
    b2 = np.asarray(inputs["b2"], np.float32)
    b3 = np.asarray(inputs["b3"], np.float32)
    b4 = np.asarray(inputs["b4"], np.float32)
    nb1 = np.asarray(inputs["nb1"], np.float32)
    nb2 = np.asarray(inputs["nb2"], np.float32)

    gumb = _gumbel_np()
    selw_fold = np.ascontiguousarray((sel_w @ W4.T).T)      # [768, 128]

    shared = {
        "w1": np.ascontiguousarray(np.tile(W1, (4, 1))),
        "w2": _ktile_wide(W2),
        "w3": _ktile_wide(W3),
        "w4": _ktile_wide(W4),
        "selw": _ktile_wide(selw_fold),
        "nw1": _ktile_wide(nW1),
        "nw2": _ktile_wide(nW2),
        "b1c": np.ascontiguousarray(b1.reshape(2, 128).T),
        "b2c": np.ascontiguousarray(b2.reshape(4, 128).T),
        "b3c": np.ascontiguousarray(b3.reshape(6, 128).T),
        "b4row": np.ascontiguousarray(b4.reshape(1, -1)),
        "nb1row": np.ascontiguousarray(nb1.reshape(1, -1)),
        "nb2row": np.ascontiguousarray(nb2.reshape(1, -1)),
        "ident": np.eye(128, dtype=np.float32),
        "iota8t": np.tile(np.arange(8, dtype=np.float32), (128, 1)),
        "iota128t": np.tile(np.arange(128, dtype=np.float32), (128, 1)),
        "lt128": np.tril(np.ones((128, 128), np.float32), -1),
    }

    in_maps = []
    q = M // 4
    for b in range(B):
        seg = slice(b * M, (b + 1) * M)
        xyz = coordinates[seg, 1:4]
        c4 = np.concatenate(
            [xyz.T, (-0.5 * np.sum(xyz * xyz, axis=1))[None, :]], axis=0)
        fT = features[seg].T                       # [32, M]
        fpk = np.concatenate([fT[:, g * q:(g + 1) * q] for g in range(4)], 0)
        cpk = np.zeros((128, q), np.float32)
        for g in range(4):
            cpk[32 * g:32 * g + 4, :] = c4[:, g * q:(g + 1) * q]
        m = dict(shared)
        m["featsT"] = np.ascontiguousarray(fpk)
        m["coordsT4"] = cpk
        m["ptsc"] = np.ascontiguousarray(coordinates[seg, 1:5])
        m["gumb"] = np.ascontiguousarray(gumb[b])
        in_maps.append(m)
    return in_maps


def kernel(**inputs):
    from concourse import bass_utils
    nc = _build()
    in_maps = _host_prep(inputs)
    res = bass_utils.run_bass_kernel_spmd(nc, in_maps, core_ids=list(range(B)))
    tokens = np.stack([res.results[b]["otok"] for b in range(B)])
    cents = np.stack([res.results[b]["ocent"] for b in range(B)])
    mask = np.ones((B, K), dtype=bool)
    return tokens, cents, mask
